# revision 8
# baseline (speedup 1.0000x reference)
import numpy as np
from contextlib import ExitStack

import concourse.bacc as bacc
import concourse.bass as bass
import concourse.tile as tile
from concourse import mybir
from concourse.bass_utils import run_bass_kernel_spmd
import ml_dtypes

BF16NP = ml_dtypes.bfloat16

F32 = mybir.dt.float32
F32R = mybir.dt.float32r
BF16 = mybir.dt.bfloat16
I16 = mybir.dt.int16
AF = mybir.ActivationFunctionType
ALU = mybir.AluOpType
AX = mybir.AxisListType

B, N, D = 2, 2048, 1024
H, HD = 16, 64
NCORE = 8
T = B * N
TOK = 512
KT = N // 128
LN2 = 0.6931471805599453
SCHB = 16250.4
DVE_KTGS = (1, 3, 5, 7)


def build_nc():
    nc = bacc.Bacc(None, target_bir_lowering=False, num_devices=NCORE)

    xT = nc.declare_dram_parameter("xT", [128, 8, T], BF16, isOutput=False)
    wq = nc.declare_dram_parameter("wq", [128, 8, 128], BF16, isOutput=False)
    wk = nc.declare_dram_parameter("wk", [128, 8, 128], BF16, isOutput=False)
    wv = nc.declare_dram_parameter("wv", [128, 8, 128], BF16, isOutput=False)
    wpT = nc.declare_dram_parameter("wpT", [D, D], BF16, isOutput=False)
    wrsT = nc.declare_dram_parameter("wrsT", [D, 17], F32R, isOutput=False)
    wpbv = nc.declare_dram_parameter("wpbv", [D], BF16, isOutput=False)
    bq = nc.declare_dram_parameter("bq", [128], F32, isOutput=False)
    bp = nc.declare_dram_parameter("bp", [D], F32, isOutput=False)
    brs = nc.declare_dram_parameter("brs", [17], F32, isOutput=False)
    lnsp2 = nc.declare_dram_parameter("lnsp2", [2], F32, isOutput=False)
    qesp = nc.declare_dram_parameter("qesp", [128], F32, isOutput=False)
    msel2 = nc.declare_dram_parameter("msel2", [128, 2], BF16, isOutput=False)
    esel2 = nc.declare_dram_parameter("esel2", [2, 128], F32R, isOutput=False)
    ones_fr = nc.declare_dram_parameter("ones_fr", [128, 64], F32R, isOutput=False)
    ones_bf = nc.declare_dram_parameter("ones_bf", [1, 128], BF16, isOutput=False)
    ident = nc.declare_dram_parameter("ident", [128, 128], F32, isOutput=False)
    out = nc.declare_dram_parameter("out", [TOK, D], F32, isOutput=True)

    with tile.TileContext(nc) as tc, ExitStack() as ctx:
        const = ctx.enter_context(tc.tile_pool(name="const", bufs=1))
        psum = ctx.enter_context(tc.tile_pool(name="psum", bufs=2, space="PSUM"))
        dram = ctx.enter_context(tc.tile_pool(name="dram", bufs=1, space="DRAM"))
        work = ctx.enter_context(tc.tile_pool(name="work", bufs=1))
        kv = ctx.enter_context(tc.tile_pool(name="kv", bufs=2))

        warm_in = dram.tile([512], BF16, name="warm_in")
        warm_out = dram.tile([8 * 512], BF16, name="warm_out")
        nc.gpsimd.collective_compute(
            "AllGather", ALU.bypass, replica_groups=[list(range(NCORE))],
            ins=[warm_in.opt()], outs=[warm_out.opt()])

        xc_tiles = []
        for j in range(8):
            xc = kv.tile([128, 8, 512], BF16, tag="xc", bufs=4, name=f"xc{j}")
            nc.sync.dma_start(out=xc, in_=xT[:, :, 512 * j:512 * j + 512])
            xc_tiles.append(xc)

        wq_sb = const.tile([128, 8, 128], BF16)
        wk_sb = const.tile([128, 8, 128], BF16)
        wv_sb = const.tile([128, 8, 128], BF16)
        nc.gpsimd.dma_start(out=wq_sb, in_=wq[:, :, :])
        nc.gpsimd.dma_start(out=wk_sb, in_=wk[:, :, :])
        nc.gpsimd.dma_start(out=wv_sb, in_=wv[:, :, :])
        bq_sb = const.tile([128, 1], F32)
        nc.gpsimd.dma_start(out=bq_sb, in_=bq[:, None])
        qe_sp = const.tile([128, 1], F32)
        nc.gpsimd.dma_start(out=qe_sp, in_=qesp[:, None])
        ln_sp = const.tile([2, 1], F32)
        nc.gpsimd.dma_start(out=ln_sp, in_=lnsp2[:, None])
        msel2_sb = const.tile([128, 2], BF16)
        nc.gpsimd.dma_start(out=msel2_sb, in_=msel2[:, :])
        esel2_sb = const.tile([2, 128], F32R)
        nc.gpsimd.dma_start(out=esel2_sb, in_=esel2[:, :])
        ones_fr_sb = const.tile([128, 64], F32R)
        nc.gpsimd.dma_start(out=ones_fr_sb, in_=ones_fr[:, :])
        ones_bf_sb = const.tile([1, 128], BF16)
        nc.gpsimd.dma_start(out=ones_bf_sb, in_=ones_bf[:, :])

        k_sb = work.tile([128, T], BF16)
        qs_sb = work.tile([128, T], BF16)
        v_sb = work.tile([128, 32, 130], BF16)
        nc.vector.memset(v_sb[:, :, 64:65], 1.0)
        nc.vector.memset(v_sb[:, :, 129:130], 1.0)
        h16 = work.tile([128, 8, 512], BF16)
        hT16 = work.tile([128, 8, 512], BF16)
        hn_all = work.tile([64, 16, 512], F32R)
        den16 = work.tile([40, 512], F32)
        den16r = work.tile([40, 512], F32R)
        den4 = work.tile([128, 4, 512], F32R)

        def proj_qt(qt):
            tsl = slice(512 * qt, 512 * qt + 512)
            xc = xc_tiles[qt]
            pk = psum.tile([128, 512], F32, tag="mm", name=f"pk{qt}")
            for s in range(8):
                nc.tensor.matmul(pk, wk_sb[:, s, :], xc[:, s, :],
                                 start=(s == 0), stop=(s == 7))
            nc.scalar.copy(k_sb[:, tsl], pk)
            for m in range(4):
                pv_ = psum.tile([128, 128], F32, tag="pv", bufs=2,
                                name=f"pv{qt}_{m}")
                for s in range(8):
                    nc.tensor.matmul(pv_, xc[:, s, 128 * m:128 * m + 128],
                                     wv_sb[:, s, :],
                                     start=(s == 0), stop=(s == 7))
                dst = v_sb[:, 4 * qt + m, :].rearrange(
                    "p (h x) -> p h x", h=2)[:, :, 0:64]
                nc.scalar.copy(dst, pv_.rearrange("p (h x) -> p h x", h=2))
            pq = psum.tile([128, 512], F32, tag="mm", name=f"pq{qt}")
            for s in range(8):
                nc.tensor.matmul(pq, wq_sb[:, s, :], xc[:, s, :],
                                 start=(s == 0), stop=(s == 7))
            qf = kv.tile([128, 512], BF16, tag="qf", name=f"qf{qt}")
            nc.vector.tensor_scalar(qf, pq, bq_sb[:, 0:1], None, ALU.add)
            sq = kv.tile([128, 512], BF16, tag="sq", name=f"sq{qt}")
            nc.vector.tensor_mul(sq, qf, qf)
            pss = psum.tile([2, 512], F32, tag="pv", bufs=2, name=f"pss{qt}")
            nc.tensor.matmul(pss, msel2_sb, sq, start=True, stop=True)
            lnss = kv.tile([2, 512], F32, tag="lnss", name=f"ln{qt}")
            nc.scalar.activation(lnss, pss, AF.Ln)
            rs = kv.tile([2, 512], F32R, tag="rs", name=f"rs{qt}")
            nc.scalar.activation(rs, lnss, AF.Exp, scale=-0.5,
                                 bias=ln_sp[:, 0:1])
            pb = psum.tile([128, 512], F32, tag="pv", bufs=2, name=f"pb{qt}")
            nc.tensor.matmul(pb, esel2_sb, rs, start=True, stop=True)
            qtmp = kv.tile([128, 512], BF16, tag="qtmp", name=f"qm{qt}")
            nc.vector.tensor_mul(qtmp, qf, pb)
            nc.vector.tensor_scalar(qs_sb[:, tsl], qtmp, qe_sp[:, 0:1], None,
                                    ALU.add)

        def attn_chunk(hh, b, qt4):
            u = 2 * b + hh
            row = 4 * u + qt4
            dpart = 32 * b + 4 * hh + qt4
            psl = slice(64 * hh, 64 * hh + 64)
            tsl = slice(2048 * b + 512 * qt4, 2048 * b + 512 * qt4 + 512)
            ppv = psum.tile([65, 512], F32, tag="ppv", name=f"ppv{row}")
            for ktg in range(8):
                ps = psum.tile([128, 2, 512], F32, tag="mm",
                               name=f"ps{row}_{ktg}")
                et = kv.tile([128, 2, 512], BF16, tag="et", bufs=4,
                             name=f"et{row}_{ktg}")
                for j in range(2):
                    kt = 2 * ktg + j
                    ksl = slice(2048 * b + 128 * kt, 2048 * b + 128 * kt + 128)
                    nc.tensor.matmul(ps[:, j, :], k_sb[psl, ksl],
                                     qs_sb[psl, tsl], start=True, stop=True)
                if ktg in DVE_KTGS:
                    nc.vector.tensor_scalar(et.bitcast(I16), ps, 128.0, SCHB,
                                            ALU.mult, ALU.add)
                else:
                    nc.scalar.activation(et, ps, AF.Exp, scale=LN2)
                for j in range(2):
                    kt = 2 * ktg + j
                    nc.tensor.matmul(ppv, v_sb[:, 16 * b + kt,
                                               65 * hh:65 * hh + 65],
                                     et[:, j, :], start=(kt == 0),
                                     stop=(kt == KT - 1))
            dstg = kv.tile([65, 512], F32, tag="dstg", name=f"ds{row}")
            nc.scalar.copy(dstg[64:65, :], ppv[64:65, :])
            nc.sync.dma_start(out=den16[dpart:dpart + 1, :],
                              in_=dstg[64:65, :])
            nc.vector.tensor_copy(hn_all[:, row, :], ppv[0:64, :])

        def finish_batch(b):
            rows = slice(32 * b, 32 * b + 8)
            dln = kv.tile([40, 512], F32, tag="dln", name=f"dln{b}")
            nc.scalar.activation(dln[rows, :], den16[rows, :], AF.Ln)
            nc.scalar.activation(den16r[rows, :], dln[rows, :], AF.Exp,
                                 scale=-1.0)
            for hh in range(2):
                for qt4 in range(4):
                    u = 2 * b + hh
                    row = 4 * u + qt4
                    dpart = 32 * b + 4 * hh + qt4
                    sh = 4 * b + qt4
                    nc.sync.dma_start(out=den4[32 * u:32 * u + 1, qt4, :],
                                      in_=den16r[dpart:dpart + 1, :])
                    pg = psum.tile([64, 512], F32, tag="pv", bufs=2,
                                   name=f"pg{row}")
                    nc.tensor.matmul(pg, ones_fr_sb[32 * u:32 * u + 1, :],
                                     den4[32 * u:32 * u + 1, qt4, :],
                                     start=True, stop=True,
                                     tile_position=(32 * u, 0))
                    if hh == 0:
                        nc.vector.tensor_mul(h16[0:64, sh, :],
                                             hn_all[:, row, :], pg)
                    else:
                        tod = kv.tile([64, 512], BF16, tag="tod",
                                      name=f"tod{row}")
                        nc.vector.tensor_mul(tod, hn_all[:, row, :], pg)
                        nc.sync.dma_start(out=h16[64:128, sh, :], in_=tod)

        for qt in range(4):
            proj_qt(qt)
        chunks_b0 = [(hh, 0, qt4) for hh in range(2) for qt4 in range(4)]
        for i, (hh, b, qt4) in enumerate(chunks_b0):
            attn_chunk(hh, b, qt4)
            if i < 4:
                proj_qt(4 + i)
        finish_batch(0)
        for hh in range(2):
            for qt4 in range(4):
                attn_chunk(hh, 1, qt4)
        finish_batch(1)

        cin = dram.tile([NCORE * 128 * 512], BF16, name="a2a_in")
        cout = dram.tile([NCORE * 128 * 512], BF16, name="a2a_out")
        nc.sync.dma_start(
            out=cin.rearrange("(j p t) -> p j t", p=128, t=512), in_=h16)
        nc.gpsimd.collective_compute(
            "AllToAll", ALU.bypass, replica_groups=[list(range(NCORE))],
            ins=[cin.opt()], outs=[cout.opt()])
        nc.sync.dma_start(
            out=hT16, in_=cout.rearrange("(j p t) -> p j t", p=128, t=512))

        bp_rep = const.tile([128, D], F32)
        nc.gpsimd.dma_start(out=bp_rep, in_=bp[None, :].to_broadcast([128, D]))
        brs_sb = const.tile([17, 1], F32)
        nc.gpsimd.dma_start(out=brs_sb, in_=brs[:, None])
        ident_sb = const.tile([128, 128], F32)
        nc.gpsimd.dma_start(out=ident_sb, in_=ident[:, :])
        wpbv_sb = const.tile([1, D], BF16)
        nc.gpsimd.dma_start(out=wpbv_sb, in_=wpbv[None, :])
        w_rs = const.tile([128, 8, 17], F32R)
        nc.gpsimd.dma_start(out=w_rs,
                            in_=wrsT.rearrange("(s p) o -> p s o", p=128))
        w_rs16 = const.tile([128, 8, 17], BF16)
        nc.vector.tensor_copy(w_rs16, w_rs)
        prs = psum.tile([17, 512], F32, tag="pv", bufs=2)
        for s in range(8):
            nc.tensor.matmul(prs, w_rs16[:, s, :], hT16[:, s, :],
                             start=(s == 0), stop=(s == 7))
        rs_sb = const.tile([17, 512], F32)
        nc.vector.tensor_tensor(rs_sb, prs,
                                brs_sb[:, 0:1].to_broadcast([17, 512]),
                                ALU.add)
        lg_t = const.tile([128, 4, 17], F32)
        for c4 in range(4):
            pt_ = psum.tile([128, 17], F32, tag="pv", bufs=2, name=f"pt{c4}")
            nc.tensor.transpose(pt_, rs_sb[:, 128 * c4:128 * c4 + 128],
                                ident_sb[0:17, 0:17])
            nc.vector.tensor_copy(lg_t[:, c4, :], pt_)

        e15 = const.tile([128, 4, 15], F32)
        nc.scalar.activation(e15, lg_t[:, :, 0:15], AF.Exp)
        e2 = const.tile([128, 4, 2], F32)
        nc.scalar.activation(e2, lg_t[:, :, 15:17], AF.Exp)
        s15 = const.tile([128, 4, 1], F32)
        nc.vector.tensor_reduce(s15, e15, AX.X, ALU.add)
        s2 = const.tile([128, 4, 1], F32)
        nc.vector.tensor_reduce(s2, e2, AX.X, ALU.add)
        m1 = const.tile([128, 4, 1], F32)
        nc.vector.tensor_reduce(m1, e15, AX.X, ALU.max)
        msk = const.tile([128, 4, 15], F32)
        nc.vector.tensor_tensor(msk, e15, m1.to_broadcast([128, 4, 15]),
                                ALU.is_ge)
        e15b = const.tile([128, 4, 15], F32)
        nc.vector.scalar_tensor_tensor(e15b, msk, -1e30, e15, ALU.mult,
                                       ALU.add)
        m2 = const.tile([128, 4, 1], F32)
        nc.vector.tensor_reduce(m2, e15b, AX.X, ALU.max)
        nc.vector.tensor_tensor(msk, e15b, m2.to_broadcast([128, 4, 15]),
                                ALU.is_ge)
        nc.vector.scalar_tensor_tensor(e15b, msk, -1e30, e15b, ALU.mult,
                                       ALU.add)
        m3 = const.tile([128, 4, 1], F32)
        nc.vector.tensor_reduce(m3, e15b, AX.X, ALU.max)
        nc.vector.tensor_add(m1, m1, m2)
        nc.vector.tensor_add(m1, m1, m3)
        nc.vector.reciprocal(s15, s15)
        nc.vector.reciprocal(s2, s2)
        ga = const.tile([128, 4, 1], F32)
        nc.vector.tensor_mul(ga, e2[:, :, 0:1], s2)
        gb = const.tile([128, 4, 1], F32)
        nc.vector.tensor_mul(gb, e2[:, :, 1:2], s2)
        nc.vector.tensor_mul(gb, gb, m1)
        nc.vector.tensor_mul(gb, gb, s15)
        nc.vector.tensor_scalar_mul(gb, gb, 6.0)
        g = const.tile([128, 4, 1], F32)
        nc.vector.scalar_tensor_tensor(g, ga, 2.0, gb, ALU.mult, ALU.add)

        for nt in range(2):
            po = [psum.tile([128, 2, 512], F32, tag="mm", name=f"po{nt}_{i}")
                  for i in range(2)]
            for mt in range(4):
                nc.tensor.matmul(po[mt // 2][:, mt % 2, :],
                                 ones_bf_sb[0:1, :],
                                 wpbv_sb[0:1, 512 * nt:512 * nt + 512],
                                 start=True, stop=False)
            for s in range(8):
                wp16 = kv.tile([128, 512], BF16, tag="wp16", bufs=4,
                               name=f"wp16_{nt}_{s}")
                nc.scalar.dma_start(
                    out=wp16, in_=wpT[128 * s:128 * s + 128,
                                      512 * nt:512 * nt + 512])
                for mt in range(4):
                    nc.tensor.matmul(
                        po[mt // 2][:, mt % 2, :],
                        hT16[:, s, 128 * mt:128 * mt + 128], wp16,
                        start=False, stop=(s == 7))
            for mt in range(4):
                ob = kv.tile([128, 512], F32, tag="ob", bufs=3,
                             name=f"ob{nt}_{mt}")
                nc.vector.tensor_mul(ob, po[mt // 2][:, mt % 2, :],
                                     g[:, mt, 0:1].to_broadcast([128, 512]))
                nc.vector.tensor_add(ob, ob, bp_rep[:, 512 * nt:512 * nt + 512])
                nc.sync.dma_start(
                    out=out[128 * mt:128 * mt + 128, 512 * nt:512 * nt + 512],
                    in_=ob)

    nc.compile()
    return nc


_NC_CACHE = {}


def _get_nc():
    if "nc" not in _NC_CACHE:
        _NC_CACHE["nc"] = build_nc()
    return _NC_CACHE["nc"]


def _host_prep(x, Wq, bq, Wk, bk, Wv, bv, Wp, bp, Wr, br, Ws, bs,
               temperature, query_embedding):
    f32 = np.float32
    xf = np.ascontiguousarray(x, dtype=f32).reshape(T, D)
    xT = np.ascontiguousarray(
        xf.T.reshape(8, 128, T).transpose(1, 0, 2)).astype(BF16NP)

    Wq_, Wk_, Wv_ = (np.asarray(W, f32) for W in (Wq, Wk, Wv))
    Wp_, Wr_, Ws_ = np.asarray(Wp, f32), np.asarray(Wr, f32), np.asarray(Ws, f32)
    bv_ = np.asarray(bv, f32)
    wrs_full = np.concatenate([Wr_, Ws_], 0)
    msel2 = np.zeros((128, 2), BF16NP)
    esel2 = np.zeros((2, 128), f32)
    for hh in range(2):
        msel2[64 * hh:64 * hh + 64, hh] = 1.0
        esel2[hh, 64 * hh:64 * hh + 64] = 1.0

    shared = {
        "xT": xT,
        "wpT": np.ascontiguousarray(Wp_.T.astype(BF16NP)),
        "wrsT": np.ascontiguousarray(wrs_full.T),
        "wpbv": np.ascontiguousarray((Wp_ @ bv_).astype(BF16NP)),
        "bp": np.ascontiguousarray(bp, f32),
        "brs": np.ascontiguousarray(
            np.concatenate([np.asarray(br, f32), np.asarray(bs, f32)])
            + wrs_full @ bv_),
        "msel2": msel2,
        "esel2": esel2,
        "ones_fr": np.ones((128, 64), f32),
        "ones_bf": np.ones((1, 128), BF16NP),
        "ident": np.eye(128, dtype=f32),
    }
    qe_full = np.asarray(query_embedding, f32).reshape(H, HD)
    temp_full = np.asarray(temperature, np.float64).reshape(H)
    spp_full = (np.log1p(np.exp(temp_full))
                * 0.125 * 1.4426950408889634).astype(f32)
    bq_full = np.asarray(bq, f32)

    def _wslice(W, c):
        ws = W[128 * c:128 * c + 128, :].T
        return np.ascontiguousarray(
            ws.reshape(8, 128, 128).transpose(1, 0, 2)).astype(BF16NP)

    in_maps = []
    for c in range(NCORE):
        m = dict(shared)
        m["wq"] = _wslice(Wq_, c)
        m["wk"] = _wslice(Wk_, c)
        m["wv"] = _wslice(Wv_, c)
        m["bq"] = np.ascontiguousarray(bq_full[128 * c:128 * c + 128])
        m["qesp"] = np.ascontiguousarray(
            (qe_full[2 * c:2 * c + 2]
             * spp_full[2 * c:2 * c + 2, None]).reshape(128))
        m["lnsp2"] = np.ascontiguousarray(
            np.log(spp_full[2 * c:2 * c + 2]))
        in_maps.append(m)
    return in_maps


def kernel(**inputs):
    nc = _get_nc()
    in_maps = _host_prep(**inputs)
    res = run_bass_kernel_spmd(nc, in_maps, core_ids=list(range(NCORE)))
    shards = [res.results[c]["out"] for c in range(NCORE)]
    return np.concatenate(shards, 0).reshape(B, N, D)


# revision 15
# speedup vs baseline: 1.2621x; 1.2621x over previous
import numpy as np
from contextlib import ExitStack

import concourse.bacc as bacc
import concourse.bass as bass
import concourse.tile as tile
from concourse import mybir
from concourse.bass_utils import run_bass_kernel_spmd
import ml_dtypes

BF16NP = ml_dtypes.bfloat16

F32 = mybir.dt.float32
F32R = mybir.dt.float32r
BF16 = mybir.dt.bfloat16
I16 = mybir.dt.int16
AF = mybir.ActivationFunctionType
ALU = mybir.AluOpType
AX = mybir.AxisListType

B, N, D = 2, 2048, 1024
H, HD = 16, 64
NCORE = 8
T = B * N
TOK = 512
KT = N // 128
LN2 = 0.6931471805599453
SCHB = 16250.4
DVE_KTGS = (1, 3, 5, 7)


def build_nc():
    nc = bacc.Bacc(None, target_bir_lowering=False, num_devices=NCORE)

    xT = nc.declare_dram_parameter("xT", [128, 8, T], BF16, isOutput=False)
    wq = nc.declare_dram_parameter("wq", [128, 8, 128], BF16, isOutput=False)
    wk = nc.declare_dram_parameter("wk", [128, 8, 128], BF16, isOutput=False)
    wv = nc.declare_dram_parameter("wv", [128, 8, 128], BF16, isOutput=False)
    wpT = nc.declare_dram_parameter("wpT", [D, D], BF16, isOutput=False)
    wrsT = nc.declare_dram_parameter("wrsT", [D, 17], F32R, isOutput=False)
    wpbv = nc.declare_dram_parameter("wpbv", [D], BF16, isOutput=False)
    bq = nc.declare_dram_parameter("bq", [128], F32, isOutput=False)
    bp = nc.declare_dram_parameter("bp", [D], F32, isOutput=False)
    brs = nc.declare_dram_parameter("brs", [17], F32, isOutput=False)
    lnsp2 = nc.declare_dram_parameter("lnsp2", [2], F32, isOutput=False)
    qesp = nc.declare_dram_parameter("qesp", [128], F32, isOutput=False)
    msel2 = nc.declare_dram_parameter("msel2", [128, 2], BF16, isOutput=False)
    esel2 = nc.declare_dram_parameter("esel2", [2, 128], BF16, isOutput=False)
    ones_fr = nc.declare_dram_parameter("ones_fr", [128, 64], F32R, isOutput=False)
    ones_bf = nc.declare_dram_parameter("ones_bf", [1, 128], BF16, isOutput=False)
    ident = nc.declare_dram_parameter("ident", [128, 128], F32, isOutput=False)
    out = nc.declare_dram_parameter("out", [TOK, D], F32, isOutput=True)

    with tile.TileContext(nc) as tc, ExitStack() as ctx:
        const = ctx.enter_context(tc.tile_pool(name="const", bufs=1))
        psum = ctx.enter_context(tc.tile_pool(name="psum", bufs=2, space="PSUM"))
        dram = ctx.enter_context(tc.tile_pool(name="dram", bufs=1, space="DRAM"))
        work = ctx.enter_context(tc.tile_pool(name="work", bufs=1))
        kv = ctx.enter_context(tc.tile_pool(name="kv", bufs=2))

        warm_in = dram.tile([512], BF16, name="warm_in")
        warm_out = dram.tile([8 * 512], BF16, name="warm_out")
        nc.gpsimd.collective_compute(
            "AllGather", ALU.bypass, replica_groups=[list(range(NCORE))],
            ins=[warm_in.opt()], outs=[warm_out.opt()])

        xc_tiles = []
        for j in range(8):
            xc = kv.tile([128, 8, 512], BF16, tag="xc", bufs=3, name=f"xc{j}")
            nc.sync.dma_start(out=xc, in_=xT[:, :, 512 * j:512 * j + 512])
            xc_tiles.append(xc)

        wq_sb = const.tile([128, 8, 128], BF16)
        wk_sb = const.tile([128, 8, 128], BF16)
        wv_sb = const.tile([128, 8, 128], BF16)
        nc.gpsimd.dma_start(out=wq_sb, in_=wq[:, :, :])
        nc.gpsimd.dma_start(out=wk_sb, in_=wk[:, :, :])
        nc.gpsimd.dma_start(out=wv_sb, in_=wv[:, :, :])
        bq_sb = const.tile([128, 1], F32)
        nc.gpsimd.dma_start(out=bq_sb, in_=bq[:, None])
        qe_sp = const.tile([128, 1], F32)
        nc.gpsimd.dma_start(out=qe_sp, in_=qesp[:, None])
        ln_sp = const.tile([2, 1], F32)
        nc.gpsimd.dma_start(out=ln_sp, in_=lnsp2[:, None])
        msel2_sb = const.tile([128, 2], BF16)
        nc.gpsimd.dma_start(out=msel2_sb, in_=msel2[:, :])
        esel2_sb = const.tile([2, 128], BF16)
        nc.gpsimd.dma_start(out=esel2_sb, in_=esel2[:, :])
        ones_fr_sb = const.tile([128, 64], F32R)
        nc.gpsimd.dma_start(out=ones_fr_sb, in_=ones_fr[:, :])
        ones_bf_sb = const.tile([1, 128], BF16)
        nc.gpsimd.dma_start(out=ones_bf_sb, in_=ones_bf[:, :])

        k_zA = work.tile([128, T], BF16)
        k_zB = work.tile([128, T], BF16)
        nc.vector.memset(k_zA[64:128, :], 0.0)
        nc.vector.memset(k_zB[0:64, :], 0.0)
        qs_sb = work.tile([128, T], BF16)
        v_sb = work.tile([128, 32, 130], BF16)
        nc.vector.memset(v_sb[:, :, 64:65], 1.0)
        nc.vector.memset(v_sb[:, :, 129:130], 1.0)
        h16 = work.tile([128, 8, 512], BF16)
        hT16 = work.tile([128, 8, 512], BF16)
        hn_all = work.tile([64, 16, 512], F32R)
        ss_sb = work.tile([2, 8, 512], BF16)
        rs_b = work.tile([2, 8, 512], BF16)
        den16 = work.tile([40, 512], F32)
        den16r = work.tile([40, 512], F32R)
        den4 = work.tile([128, 4, 512], F32R)

        qf_tiles = {}

        def proj_qt1(qt):
            tsl = slice(512 * qt, 512 * qt + 512)
            xc = xc_tiles[qt]
            pk = psum.tile([128, 512], F32, tag="mm", name=f"pk{qt}")
            for s in range(8):
                nc.tensor.matmul(pk, wk_sb[:, s, :], xc[:, s, :],
                                 start=(s == 0), stop=(s == 7))
            nc.scalar.copy(k_zA[0:64, tsl], pk[0:64, :])
            nc.scalar.copy(k_zB[64:128, tsl], pk[64:128, :])
            for m in range(4):
                pv_ = psum.tile([128, 128], F32, tag="pv", bufs=2,
                                name=f"pv{qt}_{m}")
                for s in range(8):
                    nc.tensor.matmul(pv_, xc[:, s, 128 * m:128 * m + 128],
                                     wv_sb[:, s, :],
                                     start=(s == 0), stop=(s == 7))
                dst = v_sb[:, 4 * qt + m, :].rearrange(
                    "p (h x) -> p h x", h=2)[:, :, 0:64]
                nc.scalar.copy(dst, pv_.rearrange("p (h x) -> p h x", h=2))
            pq = psum.tile([128, 512], F32, tag="mm", name=f"pq{qt}")
            for s in range(8):
                nc.tensor.matmul(pq, wq_sb[:, s, :], xc[:, s, :],
                                 start=(s == 0), stop=(s == 7))
            qf = kv.tile([128, 512], BF16, tag="qf", bufs=5, name=f"qf{qt}")
            nc.vector.tensor_scalar(qf, pq, bq_sb[:, 0:1], None, ALU.add)
            qf_tiles[qt] = qf
            sq = kv.tile([128, 512], BF16, tag="sq", name=f"sq{qt}")
            nc.vector.tensor_mul(sq, qf, qf)
            pss = psum.tile([2, 512], F32, tag="pv", bufs=2, name=f"pss{qt}")
            nc.tensor.matmul(pss, msel2_sb, sq, start=True, stop=True)
            nc.vector.tensor_copy(ss_sb[:, qt, :], pss)

        def qnorm_batch(b):
            ssl = ss_sb[:, 4 * b:4 * b + 4, :]
            nc.scalar.activation(ssl, ssl, AF.Ln)
            nc.scalar.activation(rs_b[:, 4 * b:4 * b + 4, :], ssl, AF.Exp,
                                 scale=-0.5, bias=ln_sp[:, 0:1])

        def proj_qt2(qt):
            tsl = slice(512 * qt, 512 * qt + 512)
            pb = psum.tile([128, 512], F32, tag="pv", bufs=2, name=f"pb{qt}")
            nc.tensor.matmul(pb, esel2_sb, rs_b[:, qt, :], start=True,
                             stop=True)
            qtmp = kv.tile([128, 512], BF16, tag="qtmp", name=f"qm{qt}")
            nc.vector.tensor_mul(qtmp, qf_tiles[qt], pb)
            nc.vector.tensor_scalar(qs_sb[:, tsl], qtmp, qe_sp[:, 0:1], None,
                                    ALU.add)

        def attn_chunk(hh, b, qt4):
            u = 2 * b + hh
            row = 4 * u + qt4
            dpart = 32 * b + 4 * hh + qt4
            sh = 4 * b + qt4
            k_z = k_zA if hh == 0 else k_zB
            dve_set = (1, 3, 5, 7) if row % 2 == 0 else (2, 5, 7)
            tsl = slice(2048 * b + 512 * qt4, 2048 * b + 512 * qt4 + 512)
            ppv = psum.tile([65, 512], F32, tag="ppv", name=f"ppv{row}")
            for ktg in range(8):
                ps = psum.tile([128, 2, 512], F32, tag="mm",
                               name=f"ps{row}_{ktg}")
                et = kv.tile([128, 2, 512], BF16, tag="et", bufs=3,
                             name=f"et{row}_{ktg}")
                for j in range(2):
                    kt = 2 * ktg + j
                    ksl = slice(2048 * b + 128 * kt, 2048 * b + 128 * kt + 128)
                    nc.tensor.matmul(ps[:, j, :], k_z[:, ksl],
                                     qs_sb[:, tsl], start=True, stop=True)
                if ktg in dve_set:
                    nc.vector.tensor_scalar(et.bitcast(I16), ps, 128.0, SCHB,
                                            ALU.mult, ALU.add)
                else:
                    nc.scalar.activation(et, ps, AF.Exp, scale=LN2)
                for j in range(2):
                    kt = 2 * ktg + j
                    nc.tensor.matmul(ppv, v_sb[:, 16 * b + kt,
                                               65 * hh:65 * hh + 65],
                                     et[:, j, :], start=(kt == 0),
                                     stop=(kt == KT - 1))
            dstg = kv.tile([65, 512], F32, tag="dstg", name=f"ds{row}")
            nc.scalar.copy(dstg[64:65, :], ppv[64:65, :])
            nc.sync.dma_start(out=den16[dpart:dpart + 1, :],
                              in_=dstg[64:65, :])
            if hh == 0:
                nc.vector.tensor_copy(h16[0:64, sh, :], ppv[0:64, :])
            else:
                tod = kv.tile([64, 512], BF16, tag="tod", name=f"tod{row}")
                nc.vector.tensor_copy(tod, ppv[0:64, :])
                nc.sync.dma_start(out=h16[64:128, sh, :], in_=tod)

        den_dr = dram.tile([16, 512], F32R, name="den_dr")

        def finish_batch(b):
            rows = slice(32 * b, 32 * b + 8)
            with nc.allow_low_precision(reason="f32r softmax denominator"):
                nc.vector.reciprocal(den16r[rows, :], den16[rows, :])
            nc.sync.dma_start(out=den_dr[8 * b:8 * b + 8, :],
                              in_=den16r[rows, :])
            for qt4 in range(4):
                sh = 4 * b + qt4
                pgb = kv.tile([128, 512], F32R, tag="pgb", name=f"pgb{sh}")
                for hh in range(2):
                    drow = 8 * b + 4 * hh + qt4
                    nc.sync.dma_start(
                        out=pgb[64 * hh:64 * hh + 64, :],
                        in_=den_dr[drow:drow + 1, :].to_broadcast([64, 512]))
                nc.vector.tensor_mul(h16[:, sh, :], h16[:, sh, :], pgb)

        for qt in range(4):
            proj_qt1(qt)
        qnorm_batch(0)
        for qt in range(4):
            proj_qt2(qt)
        chunks_b0 = [(hh, 0, qt4) for hh in range(2) for qt4 in range(4)]
        for i, (hh, b, qt4) in enumerate(chunks_b0):
            attn_chunk(hh, b, qt4)
            if i < 4:
                proj_qt1(4 + i)
            elif i == 4:
                qnorm_batch(1)
                for qt in range(4, 8):
                    proj_qt2(qt)
        finish_batch(0)
        for hh in range(2):
            for qt4 in range(4):
                attn_chunk(hh, 1, qt4)
        finish_batch(1)

        bp_rep = const.tile([128, D], F32)
        nc.gpsimd.dma_start(out=bp_rep, in_=bp[None, :].to_broadcast([128, D]))
        brs_sb = const.tile([17, 1], F32)
        nc.gpsimd.dma_start(out=brs_sb, in_=brs[:, None])
        ident_sb = const.tile([128, 128], F32)
        nc.gpsimd.dma_start(out=ident_sb, in_=ident[:, :])
        wpbv_sb = const.tile([1, D], BF16)
        nc.gpsimd.dma_start(out=wpbv_sb, in_=wpbv[None, :])
        w_rs = const.tile([128, 8, 17], F32R)
        nc.gpsimd.dma_start(out=w_rs,
                            in_=wrsT.rearrange("(s p) o -> p s o", p=128))
        w_rs16 = const.tile([128, 8, 17], BF16)
        nc.vector.tensor_copy(w_rs16, w_rs)
        wp16_tiles = {}
        for nt in range(2):
            for si in range(8):
                wp16 = kv.tile([128, 512], BF16, tag="wp16", bufs=16,
                               name=f"wp16_{nt}_{si}")
                nc.sync.dma_start(
                    out=wp16, in_=wpT[128 * si:128 * si + 128,
                                      512 * nt:512 * nt + 512])
                wp16_tiles[(nt, si)] = wp16

        cin = dram.tile([NCORE * 128 * 512], BF16, name="a2a_in")
        cout = dram.tile([NCORE * 128 * 512], BF16, name="a2a_out")
        nc.sync.dma_start(
            out=cin.rearrange("(j p t) -> p j t", p=128, t=512), in_=h16)
        nc.gpsimd.collective_compute(
            "AllToAll", ALU.bypass, replica_groups=[list(range(NCORE))],
            ins=[cin.opt()], outs=[cout.opt()])
        nc.sync.dma_start(
            out=hT16, in_=cout.rearrange("(j p t) -> p j t", p=128, t=512))

        prs = psum.tile([17, 512], F32, tag="pv", bufs=2)
        for s in range(8):
            nc.tensor.matmul(prs, w_rs16[:, s, :], hT16[:, s, :],
                             start=(s == 0), stop=(s == 7))
        rs_sb = const.tile([17, 512], F32)
        nc.vector.tensor_tensor(rs_sb, prs,
                                brs_sb[:, 0:1].to_broadcast([17, 512]),
                                ALU.add)
        lg_t = const.tile([128, 4, 17], F32)
        for c4 in range(4):
            pt_ = psum.tile([128, 17], F32, tag="pv", bufs=2, name=f"pt{c4}")
            nc.tensor.transpose(pt_, rs_sb[:, 128 * c4:128 * c4 + 128],
                                ident_sb[0:17, 0:17])
            nc.vector.tensor_copy(lg_t[:, c4, :], pt_)

        e15 = const.tile([128, 4, 15], F32)
        nc.scalar.activation(e15, lg_t[:, :, 0:15], AF.Exp)
        e2 = const.tile([128, 4, 2], F32)
        nc.scalar.activation(e2, lg_t[:, :, 15:17], AF.Exp)
        s15 = const.tile([128, 4, 1], F32)
        nc.vector.tensor_reduce(s15, e15, AX.X, ALU.add)
        s2 = const.tile([128, 4, 1], F32)
        nc.vector.tensor_reduce(s2, e2, AX.X, ALU.add)
        m1 = const.tile([128, 4, 1], F32)
        nc.vector.tensor_reduce(m1, e15, AX.X, ALU.max)
        msk = const.tile([128, 4, 15], F32)
        nc.vector.tensor_tensor(msk, e15, m1.to_broadcast([128, 4, 15]),
                                ALU.is_ge)
        e15b = const.tile([128, 4, 15], F32)
        nc.vector.scalar_tensor_tensor(e15b, msk, -1e30, e15, ALU.mult,
                                       ALU.add)
        m2 = const.tile([128, 4, 1], F32)
        nc.vector.tensor_reduce(m2, e15b, AX.X, ALU.max)
        nc.vector.tensor_tensor(msk, e15b, m2.to_broadcast([128, 4, 15]),
                                ALU.is_ge)
        nc.vector.scalar_tensor_tensor(e15b, msk, -1e30, e15b, ALU.mult,
                                       ALU.add)
        m3 = const.tile([128, 4, 1], F32)
        nc.vector.tensor_reduce(m3, e15b, AX.X, ALU.max)
        nc.vector.tensor_add(m1, m1, m2)
        nc.vector.tensor_add(m1, m1, m3)
        nc.vector.reciprocal(s15, s15)
        nc.vector.reciprocal(s2, s2)
        ga = const.tile([128, 4, 1], F32)
        nc.vector.tensor_mul(ga, e2[:, :, 0:1], s2)
        gb = const.tile([128, 4, 1], F32)
        nc.vector.tensor_mul(gb, e2[:, :, 1:2], s2)
        nc.vector.tensor_mul(gb, gb, m1)
        nc.vector.tensor_mul(gb, gb, s15)
        nc.vector.tensor_scalar_mul(gb, gb, 6.0)
        g = const.tile([128, 4, 1], F32)
        nc.vector.scalar_tensor_tensor(g, ga, 2.0, gb, ALU.mult, ALU.add)

        for nt in range(2):
            po = [psum.tile([128, 2, 512], F32, tag="mm", name=f"po{nt}_{i}")
                  for i in range(2)]
            for mt in range(4):
                nc.tensor.matmul(po[mt // 2][:, mt % 2, :],
                                 ones_bf_sb[0:1, :],
                                 wpbv_sb[0:1, 512 * nt:512 * nt + 512],
                                 start=True, stop=False)
            for s in range(8):
                wp16 = wp16_tiles[(nt, s)]
                for mt in range(4):
                    nc.tensor.matmul(
                        po[mt // 2][:, mt % 2, :],
                        hT16[:, s, 128 * mt:128 * mt + 128], wp16,
                        start=False, stop=(s == 7))
            for mt in range(4):
                ob = kv.tile([128, 512], F32, tag="ob", bufs=2,
                             name=f"ob{nt}_{mt}")
                nc.vector.tensor_mul(ob, po[mt // 2][:, mt % 2, :],
                                     g[:, mt, 0:1].to_broadcast([128, 512]))
                nc.vector.tensor_add(ob, ob, bp_rep[:, 512 * nt:512 * nt + 512])
                nc.sync.dma_start(
                    out=out[128 * mt:128 * mt + 128, 512 * nt:512 * nt + 512],
                    in_=ob)

    nc.compile()
    return nc


_NC_CACHE = {}


def _get_nc():
    if "nc" not in _NC_CACHE:
        _NC_CACHE["nc"] = build_nc()
    return _NC_CACHE["nc"]


def _host_prep(x, Wq, bq, Wk, bk, Wv, bv, Wp, bp, Wr, br, Ws, bs,
               temperature, query_embedding):
    f32 = np.float32
    xf = np.ascontiguousarray(x, dtype=f32).reshape(T, D)
    xT = np.ascontiguousarray(
        xf.T.reshape(8, 128, T).transpose(1, 0, 2)).astype(BF16NP)

    Wq_, Wk_, Wv_ = (np.asarray(W, f32) for W in (Wq, Wk, Wv))
    Wp_, Wr_, Ws_ = np.asarray(Wp, f32), np.asarray(Wr, f32), np.asarray(Ws, f32)
    bv_ = np.asarray(bv, f32)
    wrs_full = np.concatenate([Wr_, Ws_], 0)
    msel2 = np.zeros((128, 2), BF16NP)
    esel2 = np.zeros((2, 128), BF16NP)
    for hh in range(2):
        msel2[64 * hh:64 * hh + 64, hh] = 1.0
        esel2[hh, 64 * hh:64 * hh + 64] = 1.0

    shared = {
        "xT": xT,
        "wpT": np.ascontiguousarray(Wp_.T.astype(BF16NP)),
        "wrsT": np.ascontiguousarray(wrs_full.T),
        "wpbv": np.ascontiguousarray((Wp_ @ bv_).astype(BF16NP)),
        "bp": np.ascontiguousarray(bp, f32),
        "brs": np.ascontiguousarray(
            np.concatenate([np.asarray(br, f32), np.asarray(bs, f32)])
            + wrs_full @ bv_),
        "msel2": msel2,
        "esel2": esel2,
        "ones_fr": np.ones((128, 64), f32),
        "ones_bf": np.ones((1, 128), BF16NP),
        "ident": np.eye(128, dtype=f32),
    }
    qe_full = np.asarray(query_embedding, f32).reshape(H, HD)
    temp_full = np.asarray(temperature, np.float64).reshape(H)
    spp_full = (np.log1p(np.exp(temp_full))
                * 0.125 * 1.4426950408889634).astype(f32)
    bq_full = np.asarray(bq, f32)

    def _wslice(W, c):
        ws = W[128 * c:128 * c + 128, :].T
        return np.ascontiguousarray(
            ws.reshape(8, 128, 128).transpose(1, 0, 2)).astype(BF16NP)

    in_maps = []
    for c in range(NCORE):
        m = dict(shared)
        m["wq"] = _wslice(Wq_, c)
        m["wk"] = _wslice(Wk_, c)
        m["wv"] = _wslice(Wv_, c)
        m["bq"] = np.ascontiguousarray(bq_full[128 * c:128 * c + 128])
        m["qesp"] = np.ascontiguousarray(
            (qe_full[2 * c:2 * c + 2]
             * spp_full[2 * c:2 * c + 2, None]).reshape(128))
        m["lnsp2"] = np.ascontiguousarray(
            np.log(spp_full[2 * c:2 * c + 2]))
        in_maps.append(m)
    return in_maps


def kernel(**inputs):
    nc = _get_nc()
    in_maps = _host_prep(**inputs)
    res = run_bass_kernel_spmd(nc, in_maps, core_ids=list(range(NCORE)))
    shards = [res.results[c]["out"] for c in range(NCORE)]
    return np.concatenate(shards, 0).reshape(B, N, D)
